# revision 10
# baseline (speedup 1.0000x reference)
"""Trainium2 Bass kernel for nn_BasicAttention (dense transformer block).

Strategy (pure data parallel over 8 NeuronCores, batch-sharded):
  per core: B_CORE=256 batches of [N=49, DIM=384], processed in groups of
  G=10 batches (free-packed width W = G*49 = 490).

  Big GEMMs in f32r (full rate at moving>=256; also keeps the compiler's
  FWL off for the fp16 attention matmuls — FWL mis-reads partial-row fp16
  stationaries). Attention internals in fp16. Per group, split into a
  GEMM "head" and an attention "tail", emitted software-pipelined
  (head(g+1) before tail(g)) so the PE in-order queue always has
  independent work across the softmax serial chain:
    head(g): x DMAs (2 coalesced contiguous transfers), qkT GEMM
             [512 qk-feats, W], v GEMM (2-batch 64-padded stationary x
             tiles), scoresT (bias pre-seeded via identity matmul;
             per-(batch,head) fp16 matmuls packed 8-way into PE 32x64
             sub-tiles), exp (ACT), denominators (ones-matmul),
             reciprocal (DVE).
    tail(g): 1/s broadcast via selector matmul, attnT = fT * bcast (DVE),
             AV (parity-split PSUM banks, 4 waves of 2 heads), proj
             [384, W] accumulated over 8 head chunks, single output DMA.
  Host side: weight reordering, rel-pos bias gather, x transpose/pad to
  [3,128,B,64]+[3,128,B,49], output unpack with batch de-permutation.
"""

import sys

sys.path.insert(0, "/opt/trn_rl_repo")

import numpy as np

import concourse.bass as bass
import concourse.mybir as mybir
import concourse.tile as tile
from concourse.vector_clock import ScopedClock

# ---------------- problem constants (hardcoded per spec) ----------------
B = 2048
N = 49
DIM = 384
H = 8
KD = 32
D = 128
DH = H * D  # 1024
HQKV = 1536
N_CORES = 8
B_CORE = B // N_CORES  # 256
G = 10  # batches per group
NP = 64  # padded token stride (m padded 49->64 inside x tiles)
PIPELINE = True  # emit head(g+1) before tail(g)

F32R = mybir.dt.float32r
F32 = mybir.dt.float32
F16 = mybir.dt.float16

_CACHE = {}


# ---------------- Tile drain workaround ----------------
def _patched_drain_and_barrier(self, tick_clock, wait_clock):
    # walrus rejects >1 sem wait on the final SP Drain (TPB_CTRL); spread
    # the global-clock waits across single-wait SP nops instead.
    nc = self.nc
    probe = nc.sync.nop()
    wait_clock.add_sem_waits(probe.ins, ScopedClock({None: tick_clock.global_clock}))
    waits = []
    if probe.ins.sync_info and probe.ins.sync_info.on_wait:
        waits = list(probe.ins.sync_info.on_wait)
        probe.ins.sync_info.on_wait = waits[:1]
    assert self.sems is not None
    handles = list(self.sems.allocated().values())
    for w in waits[1:]:
        n = nc.sync.nop()
        n._wait_ge(handles[0], 0)
        n.ins.sync_info.on_wait = [w]
    nc.sync.drain()
    nc.all_engine_barrier()
    popped = nc._tile_sem_poison_stack.pop()
    assert popped is self._sem_poison
    nc.clear_and_free_semaphores(handles)
    nc.all_engine_barrier()


tile.TileContext._drain_and_barrier = _patched_drain_and_barrier


def _legalize_waits(nc, max_waits=1):
    """walrus on this toolchain rejects instructions carrying more than one
    sem wait; split excess waits onto preceding same-engine nops."""
    import bass_rust

    cnt = 0
    for f in nc.m.functions:
        for b in f.blocks:
            insts = b.instructions
            inserts = []
            for idx, inst in enumerate(insts):
                si = inst.sync_info
                waits = list(si.on_wait) if (si and si.on_wait) else []
                if len(waits) <= max_waits:
                    continue
                nops = []
                for w in waits[max_waits:]:
                    cnt += 1
                    nop = mybir.InstNoOp(
                        name=f"I-waitsplit-{cnt}",
                        engine=inst.engine,
                        ins=[],
                        outs=[],
                        sync_info=bass_rust.SyncInfo(on_wait=[w], on_update=[]),
                    )
                    try:
                        nc.register_instruction(nop)
                    except Exception:
                        pass
                    nops.append(nop)
                si.on_wait = waits[:max_waits]
                inserts.append((idx, nops))
            for idx, nops in reversed(inserts):
                for nop in reversed(nops):
                    insts.insert(idx, nop)
    return cnt


def _group_sizes(b_core=B_CORE):
    sizes = []
    b = 0
    while b < b_core:
        g = min(G, b_core - b)
        sizes.append(g)
        b += g
    return sizes


def _build_bass(with_qkv_bias, with_proj_bias, b_core=B_CORE):
    nc = bass.Bass()

    xt = nc.declare_dram_parameter("xt", [3, 128, b_core, NP], F32R, isOutput=False)
    xcg = nc.declare_dram_parameter("xcg", [3, 128, b_core, N], F32R, isOutput=False)
    wt = nc.declare_dram_parameter("wt", [3, 128, HQKV], F32R, isOutput=False)
    projt = nc.declare_dram_parameter("projt", [128, H, DIM], F32R, isOutput=False)
    biastrep = nc.declare_dram_parameter(
        "biastrep", [128, 4, G * N], F32R, isOutput=False
    )
    ones8 = nc.declare_dram_parameter("ones8", [128, 4, 8], F16, isOutput=False)
    sel = nc.declare_dram_parameter("sel", [8, 4, 128], F16, isOutput=False)
    ident = nc.declare_dram_parameter("ident", [128, 128], F32R, isOutput=False)
    if with_qkv_bias:
        qkvb = nc.declare_dram_parameter("qkvb", [1, HQKV], F32R, isOutput=False)
    if with_proj_bias:
        projb = nc.declare_dram_parameter("projb", [1, DIM], F32R, isOutput=False)
    outt = nc.declare_dram_parameter("outt", [3, 128, b_core, N], F32, isOutput=True)

    gsizes = _group_sizes(b_core)
    ngroups = len(gsizes)
    goff = np.cumsum([0] + gsizes)

    with tile.TileContext(nc) as tc:
        with (
            tc.tile_pool(name="weights", bufs=1) as wpool,
            tc.tile_pool(name="xin", bufs=3) as xpool,
            tc.tile_pool(name="qk", bufs=2) as qkpool,
            tc.tile_pool(name="vsb", bufs=2) as vpool,
            tc.tile_pool(name="attn", bufs=2) as apool,
            tc.tile_pool(name="av", bufs=2) as avpool,
            tc.tile_pool(name="fin", bufs=2) as fpool,
            tc.tile_pool(name="ps", bufs=8, space="PSUM") as pspool,
        ):
            # ---- resident constants ----
            wt_sb = [
                wpool.tile([128, HQKV], F32R, tag=f"wt{c}", name=f"wt{c}")
                for c in range(3)
            ]
            for c in range(3):
                nc.sync.dma_start(out=wt_sb[c][:], in_=wt[c])
            projt_sb = wpool.tile([128, H, DIM], F32R, tag="projt")
            nc.sync.dma_start(out=projt_sb[:], in_=projt[:])
            biastrep_sb = wpool.tile([128, 4, G * N], F32R, tag="biastrep")
            nc.sync.dma_start(out=biastrep_sb[:], in_=biastrep[:])
            ones8_sb = wpool.tile([128, 4, 8], F16, tag="ones8")
            nc.sync.dma_start(out=ones8_sb[:], in_=ones8[:])
            sel_sb = wpool.tile([8, 4, 128], F16, tag="sel")
            nc.sync.dma_start(out=sel_sb[:], in_=sel[:])
            ident_sb = wpool.tile([128, 128], F32R, tag="ident")
            nc.sync.dma_start(out=ident_sb[:], in_=ident[:])
            zeros_sb = wpool.tile([1, 128], F16, tag="zeros")
            nc.vector.memset(zeros_sb[:], 0.0)
            if with_qkv_bias:
                qkvb_sb = wpool.tile([1, HQKV], F32R, tag="qkvb")
                nc.sync.dma_start(out=qkvb_sb[:], in_=qkvb[:])
            if with_proj_bias:
                projb_sb = wpool.tile([1, DIM], F32R, tag="projb")
                nc.sync.dma_start(out=projb_sb[:], in_=projb[:])
            if with_qkv_bias or with_proj_bias:
                onesw_sb = wpool.tile([1, G * N], F32R, tag="onesw")
                nc.vector.memset(onesw_sb[:], 1.0)

            # per-group state carried from head(g) to tail(g)
            state = [None] * ngroups

            def head(g):
                gsz = gsizes[g]
                b0 = goff[g]
                W = gsz * N
                npair = gsz // 2

                # ---- load x group (both layouts, contiguous DRAM) ----
                xt_sb = xpool.tile([128, 3, G, NP], F32R, tag="xt", name=f"xt_{g}")
                xcg_sb = xpool.tile([128, 3, G, N], F32R, tag="xcg", name=f"xcg_{g}")
                nc.sync.dma_start(
                    out=xt_sb[:, :, :gsz, :],
                    in_=xt[:, :, b0 : b0 + gsz].rearrange("c p b m -> p c b m"),
                )
                nc.sync.dma_start(
                    out=xcg_sb[:, :, :gsz, :],
                    in_=xcg[:, :, b0 : b0 + gsz].rearrange("c p b m -> p c b m"),
                )

                # ---- qkT GEMM: psum [128, W] x4 (q heads 0-3, 4-7, k 0-3, 4-7)
                qk_sb = []
                for mc in range(4):
                    ps = pspool.tile([128, 512], F32, tag="ps")
                    for c in range(3):
                        nc.tensor.matmul(
                            ps[:, :W],
                            wt_sb[c][:, mc * 128 : (mc + 1) * 128],
                            xcg_sb[:, c, :gsz, :].rearrange("p b m -> p (b m)"),
                            start=(c == 0),
                            stop=(c == 2 and not with_qkv_bias),
                        )
                    if with_qkv_bias:
                        nc.tensor.matmul(
                            ps[:, :W],
                            qkvb_sb[:, mc * 128 : (mc + 1) * 128],
                            onesw_sb[:, :W],
                            start=False,
                            stop=True,
                        )
                    sb = qkpool.tile([128, G * N], F16, tag=f"qk{mc}")
                    nc.vector.tensor_copy(sb[:, :W], ps[:, :W])
                    qk_sb.append(sb)

                # ---- v GEMM: per batch pair, stationary x tile [128, 128]
                # (2 batches x 64-padded tokens -> out rows par*64+m), FWL ----
                v_sb = vpool.tile([128, G // 2, H, D], F16, tag="vsb")
                for j in range(npair):
                    for half in range(2):
                        ps = pspool.tile([128, 512], F32, tag="ps")
                        for c in range(3):
                            nc.tensor.matmul(
                                ps[:, :],
                                xt_sb[:, c, 2 * j : 2 * j + 2, :].rearrange(
                                    "p b m -> p (b m)"
                                ),
                                wt_sb[c][:, 512 + half * 512 : 1024 + half * 512],
                                start=(c == 0),
                                stop=(c == 2 and not with_qkv_bias),
                            )
                        if with_qkv_bias:
                            nc.tensor.matmul(
                                ps[:, :],
                                onesw_sb[:, :128],
                                qkvb_sb[:, 512 + half * 512 : 1024 + half * 512],
                                start=False,
                                stop=True,
                            )
                        nc.scalar.copy(
                            v_sb[:, j, half * 4 : half * 4 + 4, :],
                            ps[:, :],
                        )

                # ---- scoresT: bank q holds heads {q, q+4}; 8-way sub-tile
                # packing (row_grp = q*32, col_grp = par*64) ----
                sc_ps = []
                for q in range(4):
                    ps = pspool.tile([128, 512], F32, tag="ps", name=f"sc{g}_{q}")
                    nc.tensor.matmul(
                        ps[:, :W],
                        ident_sb[:],
                        biastrep_sb[:, q, :W],
                        start=True,
                        stop=False,
                    )
                    sc_ps.append(ps)
                for j in range(npair):
                    for t in range(2):  # head = q + 4*t
                        for par in range(2):  # batch parity
                            b = 2 * j + par
                            col = (2 * j + t) * N
                            for q in range(4):
                                hbase = q * 32
                                nc.tensor.matmul(
                                    sc_ps[q][par * 64 : par * 64 + N, col : col + N],
                                    qk_sb[2 + t][
                                        hbase : hbase + 32, b * N : (b + 1) * N
                                    ],
                                    qk_sb[t][hbase : hbase + 32, b * N : (b + 1) * N],
                                    start=False,
                                    stop=False,
                                    tile_position=(hbase, par * 64),
                                )
                fT = []
                for q in range(4):
                    ps = sc_ps[q]
                    # close the bank-wide accumulation group; strided columns
                    # overlap every scores sub-region so this schedules last
                    nc.tensor.matmul(
                        bass.AP(
                            tensor=ps.tensor,
                            offset=ps.offset,
                            ap=[ps.ap[0], [N, 2 * npair]],
                        ),
                        zeros_sb[:],
                        zeros_sb[:, : 2 * npair],
                        start=False,
                        stop=True,
                    )
                    f = apool.tile([128, G * N], F16, tag=f"fT{q}")
                    nc.scalar.activation(
                        f[:, :W], ps[:, :W], mybir.ActivationFunctionType.Exp
                    )
                    fT.append(f)

                # denominators: [8, W], rows 2q+par
                ps_s = pspool.tile([8, 512], F32, tag="ps")
                for q in range(4):
                    nc.tensor.matmul(
                        ps_s[:, :W],
                        ones8_sb[:, q, :],
                        fT[q][:, :W],
                        start=(q == 0),
                        stop=(q == 3),
                    )
                recip = apool.tile([8, G * N], F16, tag="recip")
                with nc.allow_low_precision(reason="softmax denominators in fp16"):
                    nc.vector.reciprocal(recip[:, :W], ps_s[:, :W])
                state[g] = (qk_sb, v_sb, fT, recip)

            def tail(g):
                gsz = gsizes[g]
                b0 = goff[g]
                W = gsz * N
                npair = gsz // 2
                _, v_sb, fT, recip = state[g]

                # normalize: bcast recip over m rows, multiply into attnT
                attnT = []
                for q in range(4):
                    ps_b = pspool.tile([128, 512], F32, tag="ps")
                    nc.tensor.matmul(
                        ps_b[:, :W], sel_sb[:, q, :], recip[:, :W], start=True, stop=True
                    )
                    a = apool.tile([128, G * N], F16, tag=f"attnT{q}")
                    nc.vector.tensor_mul(a[:, :W], fT[q][:, :W], ps_b[:, :W])
                    attnT.append(a)

                # ---- AV: avT_h [128(d), (par,j,n)]; parity-split PSUM banks
                # (concurrent row-group MMs to one bank on the same output
                # partitions are a fatal conflict), 4 waves of 2 heads ----
                avh_sb = avpool.tile([128, H, G * N], F32R, tag="avh")
                for w in range(4):
                    pss = []
                    for hh in range(2):
                        h = 2 * w + hh
                        pse = pspool.tile([128, 512], F32, tag="ps", name=f"avE{g}_{h}")
                        pso = pspool.tile([128, 512], F32, tag="ps", name=f"avO{g}_{h}")
                        for b in range(gsz):
                            par = b % 2
                            j = b // 2
                            col = (2 * j + (h // 4)) * N
                            tgt = pso if par else pse
                            nc.tensor.matmul(
                                tgt[:, j * N : (j + 1) * N],
                                v_sb[par * 64 : par * 64 + N, j, h, :],
                                attnT[h % 4][par * 64 : par * 64 + N, col : col + N],
                                start=(b == par),
                                stop=(b >= gsz - 2),
                            )
                        pss.append((pse, pso))
                    for hh in range(2):
                        h = 2 * w + hh
                        pse, pso = pss[hh]
                        half = npair * N
                        if hh % 2 == 0:
                            nc.vector.tensor_copy(avh_sb[:, h, :half], pse[:, :half])
                            nc.scalar.copy(
                                avh_sb[:, h, half : 2 * half], pso[:, :half]
                            )
                        else:
                            nc.scalar.copy(avh_sb[:, h, :half], pse[:, :half])
                            nc.vector.tensor_copy(
                                avh_sb[:, h, half : 2 * half], pso[:, :half]
                            )

                # ---- proj: finT [384, (par,j,n)] over 8 head chunks ----
                fin = fpool.tile([128, 3, G, N], F32, tag="fin")
                for mc in range(3):
                    ps = pspool.tile([128, 512], F32, tag="ps")
                    for h in range(H):
                        nc.tensor.matmul(
                            ps[:, :W],
                            projt_sb[:, h, mc * 128 : (mc + 1) * 128],
                            avh_sb[:, h, :W],
                            start=(h == 0),
                            stop=(h == 7 and not with_proj_bias),
                        )
                    if with_proj_bias:
                        nc.tensor.matmul(
                            ps[:, :W],
                            projb_sb[:, mc * 128 : (mc + 1) * 128],
                            onesw_sb[:, :W],
                            start=False,
                            stop=True,
                        )
                    if mc == 0:
                        nc.vector.tensor_copy(fin[:, mc, :gsz, :], ps[:, :W])
                    else:
                        nc.scalar.copy(fin[:, mc, :gsz, :], ps[:, :W])
                nc.sync.dma_start(
                    out=outt[:, :, b0 : b0 + gsz].rearrange("c p b m -> p c b m"),
                    in_=fin[:, :, :gsz, :],
                )

            # software pipeline: head(g+1) emitted before tail(g)
            if PIPELINE:
                head(0)
                for g in range(1, ngroups):
                    head(g)
                    tail(g - 1)
                tail(ngroups - 1)
            else:
                for g in range(ngroups):
                    head(g)
                    tail(g)

    nsplit = _legalize_waits(nc)
    if nsplit:
        print(f"[kernel] split {nsplit} excess sem waits onto nops")
    return nc


def _host_prep(x, qkv_w, qkv_b, proj_w, proj_b, attn_bias, bias_idxs):
    """Build per-core input maps."""
    scale = KD ** -0.5
    # reorder qkv weight rows: per head [q(32) k(32) v(128)] -> q_all k_all v_all
    wq = np.concatenate([qkv_w[h * 192 : h * 192 + 32] for h in range(H)], 0) * scale
    wk = np.concatenate([qkv_w[h * 192 + 32 : h * 192 + 64] for h in range(H)], 0)
    wv = np.concatenate([qkv_w[h * 192 + 64 : h * 192 + 192] for h in range(H)], 0)
    w_cat = np.concatenate([wq, wk, wv], 0)  # [1536, 384]
    wT = np.ascontiguousarray(w_cat.T)  # [384, 1536]
    wt_arr = wT.reshape(3, 128, HQKV).astype(np.float32)

    projt_arr = np.ascontiguousarray(proj_w.T).reshape(128 * H, DIM)
    projt_arr = (
        projt_arr.reshape(H, 128, DIM).transpose(1, 0, 2).astype(np.float32)
    )  # [128, H, DIM]

    bias_full = attn_bias[:, bias_idxs]  # [H, N, N] indexed (h, n, m)
    # biastrep[q]: rows par*64+m, cols (j, hp, n) -> bias[q+4*hp, n, m]
    biastrep_arr = np.zeros((128, 4, G * N), np.float32)
    for q in range(4):
        for t in range(2):
            bT = bias_full[q + 4 * t].T  # [m, n]
            for j in range(G // 2):
                for par in range(2):
                    biastrep_arr[
                        par * 64 : par * 64 + N, q, (2 * j + t) * N : (2 * j + t + 1) * N
                    ] = bT

    ones8_arr = np.zeros((128, 4, 8), np.float16)
    for q in range(4):
        for par in range(2):
            ones8_arr[par * 64 : par * 64 + N, q, 2 * q + par] = 1.0

    sel_arr = np.zeros((8, 4, 128), np.float16)
    for q in range(4):
        for par in range(2):
            sel_arr[2 * q + par, q, par * 64 : par * 64 + N] = 1.0

    ident_arr = np.eye(128, dtype=np.float32)

    # x: [B, N, DIM] -> [3, 128, B, 64] (padded) and [3, 128, B, 49] fp16
    xT = x.transpose(0, 2, 1).reshape(B, 3, 128, N).transpose(1, 2, 0, 3)
    xcg_arr = np.ascontiguousarray(xT).astype(np.float32)  # [3, 128, B, 49]
    xt_arr = np.zeros((3, 128, B, NP), np.float32)
    xt_arr[:, :, :, :N] = xcg_arr

    qb = np.concatenate([qkv_b[h * 192 : h * 192 + 32] for h in range(H)]) * scale
    kb = np.concatenate([qkv_b[h * 192 + 32 : h * 192 + 64] for h in range(H)])
    vb = np.concatenate([qkv_b[h * 192 + 64 : h * 192 + 192] for h in range(H)])
    qkvb_arr = np.concatenate([qb, kb, vb]).astype(np.float32).reshape(1, HQKV)
    projb_arr = proj_b.astype(np.float32).reshape(1, DIM)

    with_qkv_bias = bool(np.any(qkvb_arr))
    with_proj_bias = bool(np.any(projb_arr))

    in_maps = []
    for c in range(N_CORES):
        m = {
            "xt": np.ascontiguousarray(xt_arr[:, :, c * B_CORE : (c + 1) * B_CORE]),
            "xcg": np.ascontiguousarray(xcg_arr[:, :, c * B_CORE : (c + 1) * B_CORE]),
            "wt": wt_arr,
            "projt": projt_arr,
            "biastrep": biastrep_arr,
            "ones8": ones8_arr,
            "sel": sel_arr,
            "ident": ident_arr,
        }
        if with_qkv_bias:
            m["qkvb"] = qkvb_arr
        if with_proj_bias:
            m["projb"] = projb_arr
        in_maps.append(m)
    return in_maps, with_qkv_bias, with_proj_bias


def _batch_perm(b_core=B_CORE):
    """Device batch order within each group is (par, j): [0,2,4,...,1,3,5,...]."""
    perm = []
    b0 = 0
    for gsz in _group_sizes(b_core):
        npair = gsz // 2
        order = [2 * j + par for par in range(2) for j in range(npair)]
        perm.extend(b0 + o for o in order)
        b0 += gsz
    return np.asarray(perm)  # perm[i] = batch stored at device column i


def _get_runner(with_qkv_bias, with_proj_bias):
    """Build (once) a reusable jitted SPMD executable, mirroring
    concourse.bass2jax.run_bass_via_pjrt but cached for repeat timing."""
    key = (with_qkv_bias, with_proj_bias)
    if key in _CACHE:
        return _CACHE[key]

    import jax
    from jax.sharding import Mesh, PartitionSpec
    from jax.experimental.shard_map import shard_map
    from concourse.bass2jax import (
        _bass_exec_p,
        install_neuronx_cc_hook,
        partition_id_tensor,
    )

    install_neuronx_cc_hook()
    nc = _build_bass(with_qkv_bias, with_proj_bias)
    partition_name = nc.partition_id_tensor.name if nc.partition_id_tensor else None

    in_names, out_names, out_avals, zero_outs = [], [], [], []
    for alloc in nc.m.functions[0].allocations:
        if not isinstance(alloc, mybir.MemoryLocationSet):
            continue
        name = alloc.memorylocations[0].name
        if alloc.kind == "ExternalInput":
            if name != partition_name:
                in_names.append(name)
        elif alloc.kind == "ExternalOutput":
            shape = tuple(alloc.tensor_shape)
            dtype = mybir.dt.np(alloc.dtype)
            out_names.append(name)
            out_avals.append(jax.core.ShapedArray(shape, dtype))
            zero_outs.append(np.zeros(shape, dtype))
    n_params = len(in_names)
    n_outs = len(out_avals)
    all_names = in_names + out_names
    if partition_name is not None:
        all_names = all_names + [partition_name]

    def _body(*args):
        operands = list(args)
        if partition_name is not None:
            operands.append(partition_id_tensor())
        outs = _bass_exec_p.bind(
            *operands,
            out_avals=tuple(out_avals),
            in_names=tuple(all_names),
            out_names=tuple(out_names),
            lowering_input_output_aliases=(),
            sim_require_finite=True,
            sim_require_nnan=True,
            nc=nc,
        )
        return tuple(outs)

    devices = jax.devices()[:N_CORES]
    mesh = Mesh(np.asarray(devices), ("core",))
    in_specs = (PartitionSpec("core"),) * (n_params + n_outs)
    out_specs = (PartitionSpec("core"),) * n_outs
    sharded = jax.jit(
        shard_map(
            _body, mesh=mesh, in_specs=in_specs, out_specs=out_specs, check_rep=False
        ),
        keep_unused=True,
    )

    from jax.sharding import NamedSharding

    def stage(concat_arrays):
        """device_put the concatenated inputs + zero out-buffers once."""
        sh = NamedSharding(mesh, PartitionSpec("core"))
        staged = [jax.device_put(a, sh) for a in concat_arrays]
        zeros = [
            jax.device_put(
                np.zeros((N_CORES * z.shape[0], *z.shape[1:]), z.dtype), sh
            )
            for z in zero_outs
        ]
        return staged + zeros

    runner = {
        "nc": nc,
        "sharded": sharded,
        "stage": stage,
        "in_names": in_names,
        "out_names": out_names,
        "out_avals": out_avals,
        "zero_outs": zero_outs,
    }
    _CACHE[key] = runner
    return runner


def _run_device(in_maps, runner):
    concat_in = [
        np.concatenate([m[name] for m in in_maps], axis=0)
        for name in runner["in_names"]
    ]
    staged = runner["stage"](concat_in)
    out_arrs = runner["sharded"](*staged)
    return np.asarray(out_arrs[0])  # [8*3, 128, B_CORE, 49]


def kernel(**inputs):
    x = np.asarray(inputs["x"], np.float32)
    in_maps, wqb, wpb = _host_prep(
        x,
        np.asarray(inputs["qkv_w"], np.float32),
        np.asarray(inputs["qkv_b"], np.float32),
        np.asarray(inputs["proj_w"], np.float32),
        np.asarray(inputs["proj_b"], np.float32),
        np.asarray(inputs["attn_bias"], np.float32),
        np.asarray(inputs["bias_idxs"]),
    )
    runner = _get_runner(wqb, wpb)
    outt = _run_device(in_maps, runner)  # [8*3, 128, B_CORE, 49]
    outt = outt.reshape(N_CORES, 3, 128, B_CORE, N)
    perm = _batch_perm()
    inv = np.empty_like(perm)
    inv[perm] = np.arange(len(perm))
    # out[b, n, dim] with dim = c*128 + p
    out = np.empty((B, N, DIM), np.float32)
    for c_id in range(N_CORES):
        dev = outt[c_id][:, :, inv]  # [3, 128, B_CORE, 49] batch-restored
        out[c_id * B_CORE : (c_id + 1) * B_CORE] = (
            dev.transpose(2, 3, 0, 1).reshape(B_CORE, N, DIM)
        )
    return np.ascontiguousarray(out)


# revision 12
# speedup vs baseline: 1.1579x; 1.1579x over previous
"""Trainium2 Bass kernel for nn_BasicAttention (dense transformer block).

Strategy (pure data parallel over 8 NeuronCores, batch-sharded):
  per core: B_CORE=256 batches of [N=49, DIM=384], processed in groups of
  G=10 batches (free-packed width W = G*49 = 490).

  Big GEMMs in f32r (full rate at moving>=256; also keeps the compiler's
  FWL off for the fp16 attention matmuls — FWL mis-reads partial-row fp16
  stationaries). Attention internals in fp16. Per group, split into a
  GEMM "head" and an attention "tail", emitted software-pipelined
  (head(g+1) before tail(g)) so the PE in-order queue always has
  independent work across the softmax serial chain:
    head(g): x DMAs (2 coalesced contiguous transfers), qkT GEMM
             [512 qk-feats, W], v GEMM (2-batch 64-padded stationary x
             tiles), scoresT (bias pre-seeded via identity matmul;
             per-(batch,head) fp16 matmuls packed 8-way into PE 32x64
             sub-tiles), exp (ACT), denominators (ones-matmul),
             reciprocal (DVE).
    tail(g): 1/s broadcast via selector matmul, attnT = fT * bcast (DVE),
             AV (parity-split PSUM banks, 4 waves of 2 heads), proj
             [384, W] accumulated over 8 head chunks, single output DMA.
  Host side: weight reordering, rel-pos bias gather, x transpose/pad to
  [3,128,B,64]+[3,128,B,49], output unpack with batch de-permutation.
"""

import sys

sys.path.insert(0, "/opt/trn_rl_repo")

import numpy as np

import concourse.bass as bass
import concourse.mybir as mybir
import concourse.tile as tile
from concourse.vector_clock import ScopedClock

# ---------------- problem constants (hardcoded per spec) ----------------
B = 2048
N = 49
DIM = 384
H = 8
KD = 32
D = 128
DH = H * D  # 1024
HQKV = 1536
N_CORES = 8
B_CORE = B // N_CORES  # 256
G = 10  # batches per group
NP = 64  # padded token stride (m padded 49->64 inside x tiles)
PIPELINE = True  # emit head(g+1) before tail(g)

F32R = mybir.dt.float32r
F32 = mybir.dt.float32
F16 = mybir.dt.float16

_CACHE = {}


# ---------------- Tile drain workaround ----------------
def _patched_drain_and_barrier(self, tick_clock, wait_clock):
    # walrus rejects >1 sem wait on the final SP Drain (TPB_CTRL); spread
    # the global-clock waits across single-wait SP nops instead.
    nc = self.nc
    probe = nc.sync.nop()
    wait_clock.add_sem_waits(probe.ins, ScopedClock({None: tick_clock.global_clock}))
    waits = []
    if probe.ins.sync_info and probe.ins.sync_info.on_wait:
        waits = list(probe.ins.sync_info.on_wait)
        probe.ins.sync_info.on_wait = waits[:1]
    assert self.sems is not None
    handles = list(self.sems.allocated().values())
    for w in waits[1:]:
        n = nc.sync.nop()
        n._wait_ge(handles[0], 0)
        n.ins.sync_info.on_wait = [w]
    nc.sync.drain()
    nc.all_engine_barrier()
    popped = nc._tile_sem_poison_stack.pop()
    assert popped is self._sem_poison
    nc.clear_and_free_semaphores(handles)
    nc.all_engine_barrier()


tile.TileContext._drain_and_barrier = _patched_drain_and_barrier


def _legalize_waits(nc, max_waits=1):
    """walrus on this toolchain rejects instructions carrying more than one
    sem wait; split excess waits onto preceding same-engine nops."""
    import bass_rust

    cnt = 0
    for f in nc.m.functions:
        for b in f.blocks:
            insts = b.instructions
            inserts = []
            for idx, inst in enumerate(insts):
                si = inst.sync_info
                waits = list(si.on_wait) if (si and si.on_wait) else []
                if len(waits) <= max_waits:
                    continue
                nops = []
                for w in waits[max_waits:]:
                    cnt += 1
                    nop = mybir.InstNoOp(
                        name=f"I-waitsplit-{cnt}",
                        engine=inst.engine,
                        ins=[],
                        outs=[],
                        sync_info=bass_rust.SyncInfo(on_wait=[w], on_update=[]),
                    )
                    try:
                        nc.register_instruction(nop)
                    except Exception:
                        pass
                    nops.append(nop)
                si.on_wait = waits[:max_waits]
                inserts.append((idx, nops))
            for idx, nops in reversed(inserts):
                for nop in reversed(nops):
                    insts.insert(idx, nop)
    return cnt


def _group_sizes(b_core=B_CORE):
    sizes = []
    b = 0
    while b < b_core:
        g = min(G, b_core - b)
        sizes.append(g)
        b += g
    return sizes


def _build_bass(with_qkv_bias, with_proj_bias, b_core=B_CORE):
    nc = bass.Bass()

    xt = nc.declare_dram_parameter("xt", [3, 128, b_core, NP], F32R, isOutput=False)
    xcg = nc.declare_dram_parameter("xcg", [3, 128, b_core, N], F32R, isOutput=False)
    wt = nc.declare_dram_parameter("wt", [3, 128, HQKV], F32R, isOutput=False)
    projt = nc.declare_dram_parameter("projt", [128, H, DIM], F32R, isOutput=False)
    biastrep = nc.declare_dram_parameter(
        "biastrep", [128, 4, G * N], F32R, isOutput=False
    )
    ones8 = nc.declare_dram_parameter("ones8", [128, 4, 8], F16, isOutput=False)
    sel = nc.declare_dram_parameter("sel", [8, 4, 128], F16, isOutput=False)
    ident = nc.declare_dram_parameter("ident", [128, 128], F32R, isOutput=False)
    if with_qkv_bias:
        qkvb = nc.declare_dram_parameter("qkvb", [1, HQKV], F32R, isOutput=False)
    if with_proj_bias:
        projb = nc.declare_dram_parameter("projb", [1, DIM], F32R, isOutput=False)
    outt = nc.declare_dram_parameter("outt", [3, 128, b_core, N], F32, isOutput=True)

    gsizes = _group_sizes(b_core)
    ngroups = len(gsizes)
    goff = np.cumsum([0] + gsizes)

    with tile.TileContext(nc) as tc:
        with (
            tc.tile_pool(name="weights", bufs=1) as wpool,
            tc.tile_pool(name="xin", bufs=3) as xpool,
            tc.tile_pool(name="qk", bufs=2) as qkpool,
            tc.tile_pool(name="vsb", bufs=2) as vpool,
            tc.tile_pool(name="attn", bufs=2) as apool,
            tc.tile_pool(name="av", bufs=2) as avpool,
            tc.tile_pool(name="fin", bufs=2) as fpool,
            tc.tile_pool(name="ps", bufs=8, space="PSUM") as pspool,
        ):
            # ---- resident constants ----
            wt_sb = [
                wpool.tile([128, HQKV], F32R, tag=f"wt{c}", name=f"wt{c}")
                for c in range(3)
            ]
            for c in range(3):
                nc.sync.dma_start(out=wt_sb[c][:], in_=wt[c])
            projt_sb = wpool.tile([128, H, DIM], F32R, tag="projt")
            nc.sync.dma_start(out=projt_sb[:], in_=projt[:])
            biastrep_sb = wpool.tile([128, 4, G * N], F32R, tag="biastrep")
            nc.sync.dma_start(out=biastrep_sb[:], in_=biastrep[:])
            ones8_sb = wpool.tile([128, 4, 8], F16, tag="ones8")
            nc.sync.dma_start(out=ones8_sb[:], in_=ones8[:])
            sel_sb = wpool.tile([8, 4, 128], F16, tag="sel")
            nc.sync.dma_start(out=sel_sb[:], in_=sel[:])
            ident_sb = wpool.tile([128, 128], F32R, tag="ident")
            nc.sync.dma_start(out=ident_sb[:], in_=ident[:])
            zeros_sb = wpool.tile([1, 128], F16, tag="zeros")
            nc.vector.memset(zeros_sb[:], 0.0)
            if with_qkv_bias:
                qkvb_sb = wpool.tile([1, HQKV], F32R, tag="qkvb")
                nc.sync.dma_start(out=qkvb_sb[:], in_=qkvb[:])
            if with_proj_bias:
                projb_sb = wpool.tile([1, DIM], F32R, tag="projb")
                nc.sync.dma_start(out=projb_sb[:], in_=projb[:])
            if with_qkv_bias or with_proj_bias:
                onesw_sb = wpool.tile([1, G * N], F32R, tag="onesw")
                nc.vector.memset(onesw_sb[:], 1.0)

            # per-group state carried from head(g) to tail(g)
            state = [None] * ngroups

            def head(g):
                gsz = gsizes[g]
                b0 = goff[g]
                W = gsz * N
                npair = gsz // 2

                # ---- load x group (both layouts, contiguous DRAM) ----
                xt_sb = xpool.tile([128, 3, G, NP], F32R, tag="xt", name=f"xt_{g}")
                xcg_sb = xpool.tile([128, 3, G, N], F32R, tag="xcg", name=f"xcg_{g}")
                nc.sync.dma_start(
                    out=xt_sb[:, :, :gsz, :],
                    in_=xt[:, :, b0 : b0 + gsz].rearrange("c p b m -> p c b m"),
                )
                nc.sync.dma_start(
                    out=xcg_sb[:, :, :gsz, :],
                    in_=xcg[:, :, b0 : b0 + gsz].rearrange("c p b m -> p c b m"),
                )

                # ---- qkT GEMM: psum [128, W] x4 (q heads 0-3, 4-7, k 0-3, 4-7)
                qk_sb = []
                for mc in range(4):
                    ps = pspool.tile([128, 512], F32, tag="ps")
                    for c in range(3):
                        nc.tensor.matmul(
                            ps[:, :W],
                            wt_sb[c][:, mc * 128 : (mc + 1) * 128],
                            xcg_sb[:, c, :gsz, :].rearrange("p b m -> p (b m)"),
                            start=(c == 0),
                            stop=(c == 2 and not with_qkv_bias),
                        )
                    if with_qkv_bias:
                        nc.tensor.matmul(
                            ps[:, :W],
                            qkvb_sb[:, mc * 128 : (mc + 1) * 128],
                            onesw_sb[:, :W],
                            start=False,
                            stop=True,
                        )
                    sb = qkpool.tile([128, G * N], F16, tag=f"qk{mc}")
                    nc.vector.tensor_copy(sb[:, :W], ps[:, :W])
                    qk_sb.append(sb)

                # ---- v GEMM: per batch pair, stationary x tile [128, 128]
                # (2 batches x 64-padded tokens -> out rows par*64+m), FWL ----
                v_sb = vpool.tile([128, G // 2, H, D], F16, tag="vsb")
                for j in range(npair):
                    for half in range(2):
                        ps = pspool.tile([128, 512], F32, tag="ps")
                        for c in range(3):
                            nc.tensor.matmul(
                                ps[:, :],
                                xt_sb[:, c, 2 * j : 2 * j + 2, :].rearrange(
                                    "p b m -> p (b m)"
                                ),
                                wt_sb[c][:, 512 + half * 512 : 1024 + half * 512],
                                start=(c == 0),
                                stop=(c == 2 and not with_qkv_bias),
                            )
                        if with_qkv_bias:
                            nc.tensor.matmul(
                                ps[:, :],
                                onesw_sb[:, :128],
                                qkvb_sb[:, 512 + half * 512 : 1024 + half * 512],
                                start=False,
                                stop=True,
                            )
                        nc.scalar.copy(
                            v_sb[:, j, half * 4 : half * 4 + 4, :],
                            ps[:, :],
                        )

                # ---- scoresT: bank q holds heads {q, q+4}; 8-way sub-tile
                # packing (row_grp = q*32, col_grp = par*64) ----
                sc_ps = []
                for q in range(4):
                    ps = pspool.tile([128, 512], F32, tag="ps", name=f"sc{g}_{q}")
                    nc.tensor.matmul(
                        ps[:, :W],
                        ident_sb[:],
                        biastrep_sb[:, q, :W],
                        start=True,
                        stop=False,
                    )
                    sc_ps.append(ps)
                for j in range(npair):
                    for t in range(2):  # head = q + 4*t
                        for par in range(2):  # batch parity
                            b = 2 * j + par
                            col = (2 * j + t) * N
                            for q in range(4):
                                hbase = q * 32
                                nc.tensor.matmul(
                                    sc_ps[q][par * 64 : par * 64 + N, col : col + N],
                                    qk_sb[2 + t][
                                        hbase : hbase + 32, b * N : (b + 1) * N
                                    ],
                                    qk_sb[t][hbase : hbase + 32, b * N : (b + 1) * N],
                                    start=False,
                                    stop=False,
                                    tile_position=(hbase, par * 64),
                                )
                fT = []
                for q in range(4):
                    ps = sc_ps[q]
                    # close the bank-wide accumulation group; strided columns
                    # overlap every scores sub-region so this schedules last
                    nc.tensor.matmul(
                        bass.AP(
                            tensor=ps.tensor,
                            offset=ps.offset,
                            ap=[ps.ap[0], [N, 2 * npair]],
                        ),
                        zeros_sb[:],
                        zeros_sb[:, : 2 * npair],
                        start=False,
                        stop=True,
                    )
                    f = apool.tile([128, G * N], F16, tag=f"fT{q}")
                    nc.scalar.activation(
                        f[:, :W], ps[:, :W], mybir.ActivationFunctionType.Exp
                    )
                    fT.append(f)
                state[g] = [qk_sb, v_sb, fT, None]

            def head_b(g):
                # denominators + reciprocal, emitted after tail(g-1) so the
                # PE reaches these only once the exps have long finished
                gsz = gsizes[g]
                W = gsz * N
                fT = state[g][2]
                ps_s = pspool.tile([8, 512], F32, tag="ps")
                for q in range(4):
                    nc.tensor.matmul(
                        ps_s[:, :W],
                        ones8_sb[:, q, :],
                        fT[q][:, :W],
                        start=(q == 0),
                        stop=(q == 3),
                    )
                recip = apool.tile([8, G * N], F16, tag="recip")
                with nc.allow_low_precision(reason="softmax denominators in fp16"):
                    nc.vector.reciprocal(recip[:, :W], ps_s[:, :W])
                state[g][3] = recip

            def tail(g):
                gsz = gsizes[g]
                b0 = goff[g]
                W = gsz * N
                npair = gsz // 2
                _, v_sb, fT, recip = state[g]

                # normalize: bcast recip over m rows, multiply into attnT
                attnT = []
                for q in range(4):
                    ps_b = pspool.tile([128, 512], F32, tag="ps")
                    nc.tensor.matmul(
                        ps_b[:, :W], sel_sb[:, q, :], recip[:, :W], start=True, stop=True
                    )
                    a = apool.tile([128, G * N], F16, tag=f"attnT{q}")
                    nc.vector.tensor_mul(a[:, :W], fT[q][:, :W], ps_b[:, :W])
                    attnT.append(a)

                # ---- AV: avT_h [128(d), (par,j,n)]; parity-split PSUM banks
                # (concurrent row-group MMs to one bank on the same output
                # partitions are a fatal conflict), 4 waves of 2 heads ----
                avh_sb = avpool.tile([128, H, G * N], F32R, tag="avh")
                for w in range(4):
                    pss = []
                    for hh in range(2):
                        h = 2 * w + hh
                        pse = pspool.tile([128, 512], F32, tag="ps", name=f"avE{g}_{h}")
                        pso = pspool.tile([128, 512], F32, tag="ps", name=f"avO{g}_{h}")
                        for b in range(gsz):
                            par = b % 2
                            j = b // 2
                            col = (2 * j + (h // 4)) * N
                            tgt = pso if par else pse
                            nc.tensor.matmul(
                                tgt[:, j * N : (j + 1) * N],
                                v_sb[par * 64 : par * 64 + N, j, h, :],
                                attnT[h % 4][par * 64 : par * 64 + N, col : col + N],
                                start=(b == par),
                                stop=(b >= gsz - 2),
                            )
                        pss.append((pse, pso))
                    for hh in range(2):
                        h = 2 * w + hh
                        pse, pso = pss[hh]
                        half = npair * N
                        if hh % 2 == 0:
                            nc.vector.tensor_copy(avh_sb[:, h, :half], pse[:, :half])
                            nc.scalar.copy(
                                avh_sb[:, h, half : 2 * half], pso[:, :half]
                            )
                        else:
                            nc.scalar.copy(avh_sb[:, h, :half], pse[:, :half])
                            nc.vector.tensor_copy(
                                avh_sb[:, h, half : 2 * half], pso[:, :half]
                            )

                # ---- proj: finT [384, (par,j,n)] over 8 head chunks ----
                fin = fpool.tile([128, 3, G, N], F32, tag="fin")
                for mc in range(3):
                    ps = pspool.tile([128, 512], F32, tag="ps")
                    for h in range(H):
                        nc.tensor.matmul(
                            ps[:, :W],
                            projt_sb[:, h, mc * 128 : (mc + 1) * 128],
                            avh_sb[:, h, :W],
                            start=(h == 0),
                            stop=(h == 7 and not with_proj_bias),
                        )
                    if with_proj_bias:
                        nc.tensor.matmul(
                            ps[:, :W],
                            projb_sb[:, mc * 128 : (mc + 1) * 128],
                            onesw_sb[:, :W],
                            start=False,
                            stop=True,
                        )
                    if mc == 0:
                        nc.vector.tensor_copy(fin[:, mc, :gsz, :], ps[:, :W])
                    else:
                        nc.scalar.copy(fin[:, mc, :gsz, :], ps[:, :W])
                nc.sync.dma_start(
                    out=outt[:, :, b0 : b0 + gsz].rearrange("c p b m -> p c b m"),
                    in_=fin[:, :, :gsz, :],
                )

            # software pipeline: head(g+1) emitted before tail(g), and
            # denom/recip of g+1 only after tail(g)
            if PIPELINE:
                head(0)
                head_b(0)
                for g in range(1, ngroups):
                    head(g)
                    tail(g - 1)
                    head_b(g)
                tail(ngroups - 1)
            else:
                for g in range(ngroups):
                    head(g)
                    head_b(g)
                    tail(g)

    nsplit = _legalize_waits(nc)
    if nsplit:
        print(f"[kernel] split {nsplit} excess sem waits onto nops")
    return nc


def _host_prep(x, qkv_w, qkv_b, proj_w, proj_b, attn_bias, bias_idxs):
    """Build per-core input maps."""
    scale = KD ** -0.5
    # reorder qkv weight rows: per head [q(32) k(32) v(128)] -> q_all k_all v_all
    wq = np.concatenate([qkv_w[h * 192 : h * 192 + 32] for h in range(H)], 0) * scale
    wk = np.concatenate([qkv_w[h * 192 + 32 : h * 192 + 64] for h in range(H)], 0)
    wv = np.concatenate([qkv_w[h * 192 + 64 : h * 192 + 192] for h in range(H)], 0)
    w_cat = np.concatenate([wq, wk, wv], 0)  # [1536, 384]
    wT = np.ascontiguousarray(w_cat.T)  # [384, 1536]
    wt_arr = wT.reshape(3, 128, HQKV).astype(np.float32)

    projt_arr = np.ascontiguousarray(proj_w.T).reshape(128 * H, DIM)
    projt_arr = (
        projt_arr.reshape(H, 128, DIM).transpose(1, 0, 2).astype(np.float32)
    )  # [128, H, DIM]

    bias_full = attn_bias[:, bias_idxs]  # [H, N, N] indexed (h, n, m)
    # biastrep[q]: rows par*64+m, cols (j, hp, n) -> bias[q+4*hp, n, m]
    biastrep_arr = np.zeros((128, 4, G * N), np.float32)
    for q in range(4):
        for t in range(2):
            bT = bias_full[q + 4 * t].T  # [m, n]
            for j in range(G // 2):
                for par in range(2):
                    biastrep_arr[
                        par * 64 : par * 64 + N, q, (2 * j + t) * N : (2 * j + t + 1) * N
                    ] = bT

    ones8_arr = np.zeros((128, 4, 8), np.float16)
    for q in range(4):
        for par in range(2):
            ones8_arr[par * 64 : par * 64 + N, q, 2 * q + par] = 1.0

    sel_arr = np.zeros((8, 4, 128), np.float16)
    for q in range(4):
        for par in range(2):
            sel_arr[2 * q + par, q, par * 64 : par * 64 + N] = 1.0

    ident_arr = np.eye(128, dtype=np.float32)

    # x: [B, N, DIM] -> [3, 128, B, 64] (padded) and [3, 128, B, 49] fp16
    xT = x.transpose(0, 2, 1).reshape(B, 3, 128, N).transpose(1, 2, 0, 3)
    xcg_arr = np.ascontiguousarray(xT).astype(np.float32)  # [3, 128, B, 49]
    xt_arr = np.zeros((3, 128, B, NP), np.float32)
    xt_arr[:, :, :, :N] = xcg_arr

    qb = np.concatenate([qkv_b[h * 192 : h * 192 + 32] for h in range(H)]) * scale
    kb = np.concatenate([qkv_b[h * 192 + 32 : h * 192 + 64] for h in range(H)])
    vb = np.concatenate([qkv_b[h * 192 + 64 : h * 192 + 192] for h in range(H)])
    qkvb_arr = np.concatenate([qb, kb, vb]).astype(np.float32).reshape(1, HQKV)
    projb_arr = proj_b.astype(np.float32).reshape(1, DIM)

    with_qkv_bias = bool(np.any(qkvb_arr))
    with_proj_bias = bool(np.any(projb_arr))

    in_maps = []
    for c in range(N_CORES):
        m = {
            "xt": np.ascontiguousarray(xt_arr[:, :, c * B_CORE : (c + 1) * B_CORE]),
            "xcg": np.ascontiguousarray(xcg_arr[:, :, c * B_CORE : (c + 1) * B_CORE]),
            "wt": wt_arr,
            "projt": projt_arr,
            "biastrep": biastrep_arr,
            "ones8": ones8_arr,
            "sel": sel_arr,
            "ident": ident_arr,
        }
        if with_qkv_bias:
            m["qkvb"] = qkvb_arr
        if with_proj_bias:
            m["projb"] = projb_arr
        in_maps.append(m)
    return in_maps, with_qkv_bias, with_proj_bias


def _batch_perm(b_core=B_CORE):
    """Device batch order within each group is (par, j): [0,2,4,...,1,3,5,...]."""
    perm = []
    b0 = 0
    for gsz in _group_sizes(b_core):
        npair = gsz // 2
        order = [2 * j + par for par in range(2) for j in range(npair)]
        perm.extend(b0 + o for o in order)
        b0 += gsz
    return np.asarray(perm)  # perm[i] = batch stored at device column i


def _get_runner(with_qkv_bias, with_proj_bias):
    """Build (once) a reusable jitted SPMD executable, mirroring
    concourse.bass2jax.run_bass_via_pjrt but cached for repeat timing."""
    key = (with_qkv_bias, with_proj_bias)
    if key in _CACHE:
        return _CACHE[key]

    import jax
    from jax.sharding import Mesh, PartitionSpec
    from jax.experimental.shard_map import shard_map
    from concourse.bass2jax import (
        _bass_exec_p,
        install_neuronx_cc_hook,
        partition_id_tensor,
    )

    install_neuronx_cc_hook()
    nc = _build_bass(with_qkv_bias, with_proj_bias)
    partition_name = nc.partition_id_tensor.name if nc.partition_id_tensor else None

    in_names, out_names, out_avals, zero_outs = [], [], [], []
    for alloc in nc.m.functions[0].allocations:
        if not isinstance(alloc, mybir.MemoryLocationSet):
            continue
        name = alloc.memorylocations[0].name
        if alloc.kind == "ExternalInput":
            if name != partition_name:
                in_names.append(name)
        elif alloc.kind == "ExternalOutput":
            shape = tuple(alloc.tensor_shape)
            dtype = mybir.dt.np(alloc.dtype)
            out_names.append(name)
            out_avals.append(jax.core.ShapedArray(shape, dtype))
            zero_outs.append(np.zeros(shape, dtype))
    n_params = len(in_names)
    n_outs = len(out_avals)
    all_names = in_names + out_names
    if partition_name is not None:
        all_names = all_names + [partition_name]

    def _body(*args):
        operands = list(args)
        if partition_name is not None:
            operands.append(partition_id_tensor())
        outs = _bass_exec_p.bind(
            *operands,
            out_avals=tuple(out_avals),
            in_names=tuple(all_names),
            out_names=tuple(out_names),
            lowering_input_output_aliases=(),
            sim_require_finite=True,
            sim_require_nnan=True,
            nc=nc,
        )
        return tuple(outs)

    devices = jax.devices()[:N_CORES]
    mesh = Mesh(np.asarray(devices), ("core",))
    in_specs = (PartitionSpec("core"),) * (n_params + n_outs)
    out_specs = (PartitionSpec("core"),) * n_outs
    sharded = jax.jit(
        shard_map(
            _body, mesh=mesh, in_specs=in_specs, out_specs=out_specs, check_rep=False
        ),
        keep_unused=True,
    )

    from jax.sharding import NamedSharding

    def stage(concat_arrays):
        """device_put the concatenated inputs + zero out-buffers once."""
        sh = NamedSharding(mesh, PartitionSpec("core"))
        staged = [jax.device_put(a, sh) for a in concat_arrays]
        zeros = [
            jax.device_put(
                np.zeros((N_CORES * z.shape[0], *z.shape[1:]), z.dtype), sh
            )
            for z in zero_outs
        ]
        return staged + zeros

    runner = {
        "nc": nc,
        "sharded": sharded,
        "stage": stage,
        "in_names": in_names,
        "out_names": out_names,
        "out_avals": out_avals,
        "zero_outs": zero_outs,
    }
    _CACHE[key] = runner
    return runner


def _run_device(in_maps, runner):
    concat_in = [
        np.concatenate([m[name] for m in in_maps], axis=0)
        for name in runner["in_names"]
    ]
    staged = runner["stage"](concat_in)
    out_arrs = runner["sharded"](*staged)
    return np.asarray(out_arrs[0])  # [8*3, 128, B_CORE, 49]


def kernel(**inputs):
    x = np.asarray(inputs["x"], np.float32)
    in_maps, wqb, wpb = _host_prep(
        x,
        np.asarray(inputs["qkv_w"], np.float32),
        np.asarray(inputs["qkv_b"], np.float32),
        np.asarray(inputs["proj_w"], np.float32),
        np.asarray(inputs["proj_b"], np.float32),
        np.asarray(inputs["attn_bias"], np.float32),
        np.asarray(inputs["bias_idxs"]),
    )
    runner = _get_runner(wqb, wpb)
    outt = _run_device(in_maps, runner)  # [8*3, 128, B_CORE, 49]
    outt = outt.reshape(N_CORES, 3, 128, B_CORE, N)
    perm = _batch_perm()
    inv = np.empty_like(perm)
    inv[perm] = np.arange(len(perm))
    # out[b, n, dim] with dim = c*128 + p
    out = np.empty((B, N, DIM), np.float32)
    for c_id in range(N_CORES):
        dev = outt[c_id][:, :, inv]  # [3, 128, B_CORE, 49] batch-restored
        out[c_id * B_CORE : (c_id + 1) * B_CORE] = (
            dev.transpose(2, 3, 0, 1).reshape(B_CORE, N, DIM)
        )
    return np.ascontiguousarray(out)


# revision 13
# speedup vs baseline: 1.3419x; 1.1589x over previous
"""Trainium2 Bass kernel for nn_BasicAttention (dense transformer block).

Strategy (pure data parallel over 8 NeuronCores, batch-sharded):
  per core: B_CORE=256 batches of [N=49, DIM=384], processed in groups of
  G=10 batches (free-packed width W = G*49 = 490).

  Big GEMMs in f32r (full rate at moving>=256; also keeps the compiler's
  FWL off for the fp16 attention matmuls — FWL mis-reads partial-row fp16
  stationaries). Attention internals in fp16. Per group, split into a
  GEMM "head" and an attention "tail", emitted software-pipelined
  (head(g+1) before tail(g)) so the PE in-order queue always has
  independent work across the softmax serial chain:
    head(g): x DMAs (2 coalesced contiguous transfers), qkT GEMM
             [512 qk-feats, W], v GEMM (2-batch 64-padded stationary x
             tiles), scoresT (bias pre-seeded via identity matmul;
             per-(batch,head) fp16 matmuls packed 8-way into PE 32x64
             sub-tiles), exp (ACT), denominators (ones-matmul),
             reciprocal (DVE).
    tail(g): 1/s broadcast via selector matmul, attnT = fT * bcast (DVE),
             AV (parity-split PSUM banks, 4 waves of 2 heads), proj
             [384, W] accumulated over 8 head chunks, single output DMA.
  Host side: weight reordering, rel-pos bias gather, x transpose/pad to
  [3,128,B,64]+[3,128,B,49], output unpack with batch de-permutation.
"""

import sys

sys.path.insert(0, "/opt/trn_rl_repo")

import numpy as np

import concourse.bass as bass
import concourse.mybir as mybir
import concourse.tile as tile
from concourse.vector_clock import ScopedClock

# ---------------- problem constants (hardcoded per spec) ----------------
B = 2048
N = 49
DIM = 384
H = 8
KD = 32
D = 128
DH = H * D  # 1024
HQKV = 1536
N_CORES = 8
B_CORE = B // N_CORES  # 256
G = 10  # batches per group
NP = 64  # padded token stride (m padded 49->64 inside x tiles)
PIPELINE = True  # emit head(g+1) before tail(g)

F32R = mybir.dt.float32r
F32 = mybir.dt.float32
F16 = mybir.dt.float16

_CACHE = {}


# ---------------- Tile drain workaround ----------------
def _patched_drain_and_barrier(self, tick_clock, wait_clock):
    # walrus rejects >1 sem wait on the final SP Drain (TPB_CTRL); spread
    # the global-clock waits across single-wait SP nops instead.
    nc = self.nc
    probe = nc.sync.nop()
    wait_clock.add_sem_waits(probe.ins, ScopedClock({None: tick_clock.global_clock}))
    waits = []
    if probe.ins.sync_info and probe.ins.sync_info.on_wait:
        waits = list(probe.ins.sync_info.on_wait)
        probe.ins.sync_info.on_wait = waits[:1]
    assert self.sems is not None
    handles = list(self.sems.allocated().values())
    for w in waits[1:]:
        n = nc.sync.nop()
        n._wait_ge(handles[0], 0)
        n.ins.sync_info.on_wait = [w]
    nc.sync.drain()
    nc.all_engine_barrier()
    popped = nc._tile_sem_poison_stack.pop()
    assert popped is self._sem_poison
    nc.clear_and_free_semaphores(handles)
    nc.all_engine_barrier()


tile.TileContext._drain_and_barrier = _patched_drain_and_barrier


def _legalize_waits(nc, max_waits=1):
    """walrus on this toolchain rejects instructions carrying more than one
    sem wait; split excess waits onto preceding same-engine nops."""
    import bass_rust

    cnt = 0
    for f in nc.m.functions:
        for b in f.blocks:
            insts = b.instructions
            inserts = []
            for idx, inst in enumerate(insts):
                si = inst.sync_info
                waits = list(si.on_wait) if (si and si.on_wait) else []
                if len(waits) <= max_waits:
                    continue
                nops = []
                for w in waits[max_waits:]:
                    cnt += 1
                    nop = mybir.InstNoOp(
                        name=f"I-waitsplit-{cnt}",
                        engine=inst.engine,
                        ins=[],
                        outs=[],
                        sync_info=bass_rust.SyncInfo(on_wait=[w], on_update=[]),
                    )
                    try:
                        nc.register_instruction(nop)
                    except Exception:
                        pass
                    nops.append(nop)
                si.on_wait = waits[:max_waits]
                inserts.append((idx, nops))
            for idx, nops in reversed(inserts):
                for nop in reversed(nops):
                    insts.insert(idx, nop)
    return cnt


def _group_sizes(b_core=B_CORE):
    sizes = []
    b = 0
    while b < b_core:
        g = min(G, b_core - b)
        sizes.append(g)
        b += g
    return sizes


def _build_bass(with_qkv_bias, with_proj_bias, b_core=B_CORE):
    nc = bass.Bass()

    xt = nc.declare_dram_parameter("xt", [3, 128, b_core, NP], F32R, isOutput=False)
    xcg = nc.declare_dram_parameter("xcg", [3, 128, b_core, N], F32R, isOutput=False)
    wt = nc.declare_dram_parameter("wt", [3, 128, HQKV], F32R, isOutput=False)
    projt = nc.declare_dram_parameter("projt", [128, H, DIM], F32R, isOutput=False)
    biastrep = nc.declare_dram_parameter(
        "biastrep", [128, 4, G * N], F32R, isOutput=False
    )
    ones8 = nc.declare_dram_parameter("ones8", [128, 4, 8], F16, isOutput=False)
    sel = nc.declare_dram_parameter("sel", [8, 4, 128], F16, isOutput=False)
    ident = nc.declare_dram_parameter("ident", [128, 128], F32R, isOutput=False)
    if with_qkv_bias:
        qkvb = nc.declare_dram_parameter("qkvb", [1, HQKV], F32R, isOutput=False)
    if with_proj_bias:
        projb = nc.declare_dram_parameter("projb", [1, DIM], F32R, isOutput=False)
    outt = nc.declare_dram_parameter("outt", [3, 128, b_core, N], F32, isOutput=True)

    gsizes = _group_sizes(b_core)
    ngroups = len(gsizes)
    goff = np.cumsum([0] + gsizes)

    with tile.TileContext(nc) as tc:
        with (
            tc.tile_pool(name="weights", bufs=1) as wpool,
            tc.tile_pool(name="xin", bufs=3) as xpool,
            tc.tile_pool(name="qk", bufs=2) as qkpool,
            tc.tile_pool(name="vsb", bufs=2) as vpool,
            tc.tile_pool(name="attn", bufs=2) as apool,
            tc.tile_pool(name="av", bufs=2) as avpool,
            tc.tile_pool(name="fin", bufs=2) as fpool,
            tc.tile_pool(name="ps", bufs=8, space="PSUM") as pspool,
        ):
            # ---- resident constants ----
            wt_sb = [
                wpool.tile([128, HQKV], F32R, tag=f"wt{c}", name=f"wt{c}")
                for c in range(3)
            ]
            for c in range(3):
                nc.sync.dma_start(out=wt_sb[c][:], in_=wt[c])
            projt_sb = wpool.tile([128, H, DIM], F32R, tag="projt")
            nc.sync.dma_start(out=projt_sb[:], in_=projt[:])
            biastrep_sb = wpool.tile([128, 4, G * N], F32R, tag="biastrep")
            nc.sync.dma_start(out=biastrep_sb[:], in_=biastrep[:])
            ones8_sb = wpool.tile([128, 4, 8], F16, tag="ones8")
            nc.sync.dma_start(out=ones8_sb[:], in_=ones8[:])
            sel_sb = wpool.tile([8, 4, 128], F16, tag="sel")
            nc.sync.dma_start(out=sel_sb[:], in_=sel[:])
            ident_sb = wpool.tile([128, 128], F32R, tag="ident")
            nc.sync.dma_start(out=ident_sb[:], in_=ident[:])
            zeros_sb = wpool.tile([1, 128], F16, tag="zeros")
            nc.vector.memset(zeros_sb[:], 0.0)
            if with_qkv_bias:
                qkvb_sb = wpool.tile([1, HQKV], F32R, tag="qkvb")
                nc.sync.dma_start(out=qkvb_sb[:], in_=qkvb[:])
            if with_proj_bias:
                projb_sb = wpool.tile([1, DIM], F32R, tag="projb")
                nc.sync.dma_start(out=projb_sb[:], in_=projb[:])
            if with_qkv_bias or with_proj_bias:
                onesw_sb = wpool.tile([1, G * N], F32R, tag="onesw")
                nc.vector.memset(onesw_sb[:], 1.0)

            # per-group state carried across pipeline stages
            state = [None] * ngroups

            def gemm_head(g):
                gsz = gsizes[g]
                b0 = goff[g]
                W = gsz * N

                # ---- load x group (both layouts, contiguous DRAM) ----
                xt_sb = xpool.tile([128, 3, G, NP], F32R, tag="xt", name=f"xt_{g}")
                xcg_sb = xpool.tile([128, 3, G, N], F32R, tag="xcg", name=f"xcg_{g}")
                nc.sync.dma_start(
                    out=xt_sb[:, :, :gsz, :],
                    in_=xt[:, :, b0 : b0 + gsz].rearrange("c p b m -> p c b m"),
                )
                nc.sync.dma_start(
                    out=xcg_sb[:, :, :gsz, :],
                    in_=xcg[:, :, b0 : b0 + gsz].rearrange("c p b m -> p c b m"),
                )

                # ---- qkT GEMM: psum [128, W] x4 (q heads 0-3, 4-7, k 0-3, 4-7)
                qk_sb = []
                for mc in range(4):
                    ps = pspool.tile([128, 512], F32, tag="ps")
                    for c in range(3):
                        nc.tensor.matmul(
                            ps[:, :W],
                            wt_sb[c][:, mc * 128 : (mc + 1) * 128],
                            xcg_sb[:, c, :gsz, :].rearrange("p b m -> p (b m)"),
                            start=(c == 0),
                            stop=(c == 2 and not with_qkv_bias),
                        )
                    if with_qkv_bias:
                        nc.tensor.matmul(
                            ps[:, :W],
                            qkvb_sb[:, mc * 128 : (mc + 1) * 128],
                            onesw_sb[:, :W],
                            start=False,
                            stop=True,
                        )
                    sb = qkpool.tile([128, G * N], F16, tag=f"qk{mc}")
                    nc.vector.tensor_copy(sb[:, :W], ps[:, :W])
                    qk_sb.append(sb)
                state[g] = {"qk": qk_sb, "xt": xt_sb}

            def denom_recip(g):
                # denominators + reciprocal; PE reaches this one qkT after
                # the exps were issued, so no stall
                gsz = gsizes[g]
                W = gsz * N
                fT = state[g]["fT"]
                ps_s = pspool.tile([8, 512], F32, tag="ps")
                for q in range(4):
                    nc.tensor.matmul(
                        ps_s[:, :W],
                        ones8_sb[:, q, :],
                        fT[q][:, :W],
                        start=(q == 0),
                        stop=(q == 3),
                    )
                recip = apool.tile([8, G * N], F16, tag="recip")
                with nc.allow_low_precision(reason="softmax denominators in fp16"):
                    nc.vector.reciprocal(recip[:, :W], ps_s[:, :W])
                state[g]["recip"] = recip

            def vstage(g):
                # ---- v GEMM: per batch pair, stationary x tile [128, 128]
                # (2 batches x 64-padded tokens -> out rows par*64+m) ----
                gsz = gsizes[g]
                npair = gsz // 2
                xt_sb = state[g]["xt"]
                v_sb = vpool.tile([128, G // 2, H, D], F16, tag="vsb")
                for j in range(npair):
                    for half in range(2):
                        ps = pspool.tile([128, 512], F32, tag="ps")
                        for c in range(3):
                            nc.tensor.matmul(
                                ps[:, :],
                                xt_sb[:, c, 2 * j : 2 * j + 2, :].rearrange(
                                    "p b m -> p (b m)"
                                ),
                                wt_sb[c][:, 512 + half * 512 : 1024 + half * 512],
                                start=(c == 0),
                                stop=(c == 2 and not with_qkv_bias),
                            )
                        if with_qkv_bias:
                            nc.tensor.matmul(
                                ps[:, :],
                                onesw_sb[:, :128],
                                qkvb_sb[:, 512 + half * 512 : 1024 + half * 512],
                                start=False,
                                stop=True,
                            )
                        nc.scalar.copy(
                            v_sb[:, j, half * 4 : half * 4 + 4, :],
                            ps[:, :],
                        )
                state[g]["v"] = v_sb

            def bcast_mul(g):
                # normalize: bcast recip over m rows, multiply into attnT
                gsz = gsizes[g]
                W = gsz * N
                fT, recip = state[g]["fT"], state[g]["recip"]
                attnT = []
                for q in range(4):
                    ps_b = pspool.tile([128, 512], F32, tag="ps")
                    nc.tensor.matmul(
                        ps_b[:, :W], sel_sb[:, q, :], recip[:, :W], start=True, stop=True
                    )
                    a = apool.tile([128, G * N], F16, tag=f"attnT{q}")
                    nc.vector.tensor_mul(a[:, :W], fT[q][:, :W], ps_b[:, :W])
                    attnT.append(a)
                state[g]["attnT"] = attnT

            def av_proj(g):
                # ---- AV (parity-split PSUM banks, 4 waves of 2 heads)
                # interleaved with proj partials so the PE keeps strong
                # activity through the LDW-heavy AV stretch ----
                gsz = gsizes[g]
                b0 = goff[g]
                W = gsz * N
                npair = gsz // 2
                v_sb, attnT = state[g]["v"], state[g]["attnT"]
                avh_sb = avpool.tile([128, H, G * N], F32R, tag="avh")
                fin = fpool.tile([128, 3, G, N], F32, tag="fin")
                ps_proj = [None] * 3
                for w in range(4):
                    pss = []
                    for hh in range(2):
                        h = 2 * w + hh
                        pse = pspool.tile([128, 512], F32, tag="ps", name=f"avE{g}_{h}")
                        pso = pspool.tile([128, 512], F32, tag="ps", name=f"avO{g}_{h}")
                        for b in range(gsz):
                            par = b % 2
                            j = b // 2
                            col = (2 * j + (h // 4)) * N
                            tgt = pso if par else pse
                            nc.tensor.matmul(
                                tgt[:, j * N : (j + 1) * N],
                                v_sb[par * 64 : par * 64 + N, j, h, :],
                                attnT[h % 4][par * 64 : par * 64 + N, col : col + N],
                                start=(b == par),
                                stop=(b >= gsz - 2),
                            )
                        pss.append((pse, pso))
                    for hh in range(2):
                        h = 2 * w + hh
                        pse, pso = pss[hh]
                        half = npair * N
                        if hh % 2 == 0:
                            nc.vector.tensor_copy(avh_sb[:, h, :half], pse[:, :half])
                            nc.scalar.copy(
                                avh_sb[:, h, half : 2 * half], pso[:, :half]
                            )
                        else:
                            nc.scalar.copy(avh_sb[:, h, :half], pse[:, :half])
                            nc.vector.tensor_copy(
                                avh_sb[:, h, half : 2 * half], pso[:, :half]
                            )
                    # proj partials for this wave's heads (f32r activity
                    # keeps the PE HAM-warm through the AV stretch)
                    for mc in range(3):
                        if w == 0:
                            ps_proj[mc] = pspool.tile(
                                [128, 512], F32, tag="ps", name=f"proj{g}_{mc}"
                            )
                        for hh in range(2):
                            h = 2 * w + hh
                            nc.tensor.matmul(
                                ps_proj[mc][:, :W],
                                projt_sb[:, h, mc * 128 : (mc + 1) * 128],
                                avh_sb[:, h, :W],
                                start=(h == 0),
                                stop=(h == 7 and not with_proj_bias),
                            )
                for mc in range(3):
                    if with_proj_bias:
                        nc.tensor.matmul(
                            ps_proj[mc][:, :W],
                            projb_sb[:, mc * 128 : (mc + 1) * 128],
                            onesw_sb[:, :W],
                            start=False,
                            stop=True,
                        )
                    if mc == 0:
                        nc.vector.tensor_copy(fin[:, mc, :gsz, :], ps_proj[mc][:, :W])
                    else:
                        nc.scalar.copy(fin[:, mc, :gsz, :], ps_proj[mc][:, :W])
                nc.sync.dma_start(
                    out=outt[:, :, b0 : b0 + gsz].rearrange("c p b m -> p c b m"),
                    in_=fin[:, :, :gsz, :],
                )

            def scores_exp(g):
                # ---- scoresT: bank q holds heads {q, q+4}; 8-way sub-tile
                # packing (row_grp = q*32, col_grp = par*64) ----
                gsz = gsizes[g]
                W = gsz * N
                npair = gsz // 2
                qk_sb = state[g]["qk"]
                sc_ps = []
                for q in range(4):
                    ps = pspool.tile([128, 512], F32, tag="ps", name=f"sc{g}_{q}")
                    nc.tensor.matmul(
                        ps[:, :W],
                        ident_sb[:],
                        biastrep_sb[:, q, :W],
                        start=True,
                        stop=False,
                    )
                    sc_ps.append(ps)
                for j in range(npair):
                    for t in range(2):  # head = q + 4*t
                        for par in range(2):  # batch parity
                            b = 2 * j + par
                            col = (2 * j + t) * N
                            for q in range(4):
                                hbase = q * 32
                                nc.tensor.matmul(
                                    sc_ps[q][par * 64 : par * 64 + N, col : col + N],
                                    qk_sb[2 + t][
                                        hbase : hbase + 32, b * N : (b + 1) * N
                                    ],
                                    qk_sb[t][hbase : hbase + 32, b * N : (b + 1) * N],
                                    start=False,
                                    stop=False,
                                    tile_position=(hbase, par * 64),
                                )
                fT = []
                for q in range(4):
                    ps = sc_ps[q]
                    # close the bank-wide accumulation group; strided columns
                    # overlap every scores sub-region so this schedules last
                    nc.tensor.matmul(
                        bass.AP(
                            tensor=ps.tensor,
                            offset=ps.offset,
                            ap=[ps.ap[0], [N, 2 * npair]],
                        ),
                        zeros_sb[:],
                        zeros_sb[:, : 2 * npair],
                        start=False,
                        stop=True,
                    )
                    f = apool.tile([128, G * N], F16, tag=f"fT{q}")
                    nc.scalar.activation(
                        f[:, :W], ps[:, :W], mybir.ActivationFunctionType.Exp
                    )
                    fT.append(f)
                state[g]["fT"] = fT

            # software pipeline: group g's attention tail is woven through
            # group g+1's GEMM stages to keep the PE dense and HAM-warm
            if PIPELINE:
                for g in range(ngroups):
                    gemm_head(g)
                    if g >= 1:
                        denom_recip(g - 1)
                    vstage(g)
                    if g >= 1:
                        bcast_mul(g - 1)
                        av_proj(g - 1)
                    scores_exp(g)
                denom_recip(ngroups - 1)
                bcast_mul(ngroups - 1)
                av_proj(ngroups - 1)
            else:
                for g in range(ngroups):
                    gemm_head(g)
                    vstage(g)
                    scores_exp(g)
                    denom_recip(g)
                    bcast_mul(g)
                    av_proj(g)

    nsplit = _legalize_waits(nc)
    if nsplit:
        print(f"[kernel] split {nsplit} excess sem waits onto nops")
    return nc


def _host_prep(x, qkv_w, qkv_b, proj_w, proj_b, attn_bias, bias_idxs):
    """Build per-core input maps."""
    scale = KD ** -0.5
    # reorder qkv weight rows: per head [q(32) k(32) v(128)] -> q_all k_all v_all
    wq = np.concatenate([qkv_w[h * 192 : h * 192 + 32] for h in range(H)], 0) * scale
    wk = np.concatenate([qkv_w[h * 192 + 32 : h * 192 + 64] for h in range(H)], 0)
    wv = np.concatenate([qkv_w[h * 192 + 64 : h * 192 + 192] for h in range(H)], 0)
    w_cat = np.concatenate([wq, wk, wv], 0)  # [1536, 384]
    wT = np.ascontiguousarray(w_cat.T)  # [384, 1536]
    wt_arr = wT.reshape(3, 128, HQKV).astype(np.float32)

    projt_arr = np.ascontiguousarray(proj_w.T).reshape(128 * H, DIM)
    projt_arr = (
        projt_arr.reshape(H, 128, DIM).transpose(1, 0, 2).astype(np.float32)
    )  # [128, H, DIM]

    bias_full = attn_bias[:, bias_idxs]  # [H, N, N] indexed (h, n, m)
    # biastrep[q]: rows par*64+m, cols (j, hp, n) -> bias[q+4*hp, n, m]
    biastrep_arr = np.zeros((128, 4, G * N), np.float32)
    for q in range(4):
        for t in range(2):
            bT = bias_full[q + 4 * t].T  # [m, n]
            for j in range(G // 2):
                for par in range(2):
                    biastrep_arr[
                        par * 64 : par * 64 + N, q, (2 * j + t) * N : (2 * j + t + 1) * N
                    ] = bT

    ones8_arr = np.zeros((128, 4, 8), np.float16)
    for q in range(4):
        for par in range(2):
            ones8_arr[par * 64 : par * 64 + N, q, 2 * q + par] = 1.0

    sel_arr = np.zeros((8, 4, 128), np.float16)
    for q in range(4):
        for par in range(2):
            sel_arr[2 * q + par, q, par * 64 : par * 64 + N] = 1.0

    ident_arr = np.eye(128, dtype=np.float32)

    # x: [B, N, DIM] -> [3, 128, B, 64] (padded) and [3, 128, B, 49] fp16
    xT = x.transpose(0, 2, 1).reshape(B, 3, 128, N).transpose(1, 2, 0, 3)
    xcg_arr = np.ascontiguousarray(xT).astype(np.float32)  # [3, 128, B, 49]
    xt_arr = np.zeros((3, 128, B, NP), np.float32)
    xt_arr[:, :, :, :N] = xcg_arr

    qb = np.concatenate([qkv_b[h * 192 : h * 192 + 32] for h in range(H)]) * scale
    kb = np.concatenate([qkv_b[h * 192 + 32 : h * 192 + 64] for h in range(H)])
    vb = np.concatenate([qkv_b[h * 192 + 64 : h * 192 + 192] for h in range(H)])
    qkvb_arr = np.concatenate([qb, kb, vb]).astype(np.float32).reshape(1, HQKV)
    projb_arr = proj_b.astype(np.float32).reshape(1, DIM)

    with_qkv_bias = bool(np.any(qkvb_arr))
    with_proj_bias = bool(np.any(projb_arr))

    in_maps = []
    for c in range(N_CORES):
        m = {
            "xt": np.ascontiguousarray(xt_arr[:, :, c * B_CORE : (c + 1) * B_CORE]),
            "xcg": np.ascontiguousarray(xcg_arr[:, :, c * B_CORE : (c + 1) * B_CORE]),
            "wt": wt_arr,
            "projt": projt_arr,
            "biastrep": biastrep_arr,
            "ones8": ones8_arr,
            "sel": sel_arr,
            "ident": ident_arr,
        }
        if with_qkv_bias:
            m["qkvb"] = qkvb_arr
        if with_proj_bias:
            m["projb"] = projb_arr
        in_maps.append(m)
    return in_maps, with_qkv_bias, with_proj_bias


def _batch_perm(b_core=B_CORE):
    """Device batch order within each group is (par, j): [0,2,4,...,1,3,5,...]."""
    perm = []
    b0 = 0
    for gsz in _group_sizes(b_core):
        npair = gsz // 2
        order = [2 * j + par for par in range(2) for j in range(npair)]
        perm.extend(b0 + o for o in order)
        b0 += gsz
    return np.asarray(perm)  # perm[i] = batch stored at device column i


def _get_runner(with_qkv_bias, with_proj_bias):
    """Build (once) a reusable jitted SPMD executable, mirroring
    concourse.bass2jax.run_bass_via_pjrt but cached for repeat timing."""
    key = (with_qkv_bias, with_proj_bias)
    if key in _CACHE:
        return _CACHE[key]

    import jax
    from jax.sharding import Mesh, PartitionSpec
    from jax.experimental.shard_map import shard_map
    from concourse.bass2jax import (
        _bass_exec_p,
        install_neuronx_cc_hook,
        partition_id_tensor,
    )

    install_neuronx_cc_hook()
    nc = _build_bass(with_qkv_bias, with_proj_bias)
    partition_name = nc.partition_id_tensor.name if nc.partition_id_tensor else None

    in_names, out_names, out_avals, zero_outs = [], [], [], []
    for alloc in nc.m.functions[0].allocations:
        if not isinstance(alloc, mybir.MemoryLocationSet):
            continue
        name = alloc.memorylocations[0].name
        if alloc.kind == "ExternalInput":
            if name != partition_name:
                in_names.append(name)
        elif alloc.kind == "ExternalOutput":
            shape = tuple(alloc.tensor_shape)
            dtype = mybir.dt.np(alloc.dtype)
            out_names.append(name)
            out_avals.append(jax.core.ShapedArray(shape, dtype))
            zero_outs.append(np.zeros(shape, dtype))
    n_params = len(in_names)
    n_outs = len(out_avals)
    all_names = in_names + out_names
    if partition_name is not None:
        all_names = all_names + [partition_name]

    def _body(*args):
        operands = list(args)
        if partition_name is not None:
            operands.append(partition_id_tensor())
        outs = _bass_exec_p.bind(
            *operands,
            out_avals=tuple(out_avals),
            in_names=tuple(all_names),
            out_names=tuple(out_names),
            lowering_input_output_aliases=(),
            sim_require_finite=True,
            sim_require_nnan=True,
            nc=nc,
        )
        return tuple(outs)

    devices = jax.devices()[:N_CORES]
    mesh = Mesh(np.asarray(devices), ("core",))
    in_specs = (PartitionSpec("core"),) * (n_params + n_outs)
    out_specs = (PartitionSpec("core"),) * n_outs
    sharded = jax.jit(
        shard_map(
            _body, mesh=mesh, in_specs=in_specs, out_specs=out_specs, check_rep=False
        ),
        keep_unused=True,
    )

    from jax.sharding import NamedSharding

    def stage(concat_arrays):
        """device_put the concatenated inputs + zero out-buffers once."""
        sh = NamedSharding(mesh, PartitionSpec("core"))
        staged = [jax.device_put(a, sh) for a in concat_arrays]
        zeros = [
            jax.device_put(
                np.zeros((N_CORES * z.shape[0], *z.shape[1:]), z.dtype), sh
            )
            for z in zero_outs
        ]
        return staged + zeros

    runner = {
        "nc": nc,
        "sharded": sharded,
        "stage": stage,
        "in_names": in_names,
        "out_names": out_names,
        "out_avals": out_avals,
        "zero_outs": zero_outs,
    }
    _CACHE[key] = runner
    return runner


def _run_device(in_maps, runner):
    concat_in = [
        np.concatenate([m[name] for m in in_maps], axis=0)
        for name in runner["in_names"]
    ]
    staged = runner["stage"](concat_in)
    out_arrs = runner["sharded"](*staged)
    return np.asarray(out_arrs[0])  # [8*3, 128, B_CORE, 49]


def kernel(**inputs):
    x = np.asarray(inputs["x"], np.float32)
    in_maps, wqb, wpb = _host_prep(
        x,
        np.asarray(inputs["qkv_w"], np.float32),
        np.asarray(inputs["qkv_b"], np.float32),
        np.asarray(inputs["proj_w"], np.float32),
        np.asarray(inputs["proj_b"], np.float32),
        np.asarray(inputs["attn_bias"], np.float32),
        np.asarray(inputs["bias_idxs"]),
    )
    runner = _get_runner(wqb, wpb)
    outt = _run_device(in_maps, runner)  # [8*3, 128, B_CORE, 49]
    outt = outt.reshape(N_CORES, 3, 128, B_CORE, N)
    perm = _batch_perm()
    inv = np.empty_like(perm)
    inv[perm] = np.arange(len(perm))
    # out[b, n, dim] with dim = c*128 + p
    out = np.empty((B, N, DIM), np.float32)
    for c_id in range(N_CORES):
        dev = outt[c_id][:, :, inv]  # [3, 128, B_CORE, 49] batch-restored
        out[c_id * B_CORE : (c_id + 1) * B_CORE] = (
            dev.transpose(2, 3, 0, 1).reshape(B_CORE, N, DIM)
        )
    return np.ascontiguousarray(out)
